# revision 2
# baseline (speedup 1.0000x reference)
"""Trainium2 Bass kernel v2 for nn_Net_SLSTM_Conv.

Data-parallel over T=512 on 8 cores (TC=64 t-columns per core), with TWO
software-pipelined groups of 32 columns per core to hide the recurrent-chain
latency (matmul -> sigmoid -> DVE -> sigmoid -> DVE loop).

Key restructurings vs the v1 baseline:
  - conv1d spikes via Act `Sign(conv-1)` on col-tiled [128,512] PSUM banks
    (4 t-chunks stacked on partitions -> 4x fewer activation calls); the
    {-1,+1} encoding folds into layer-1 weights (w/2) and bias (+w@1/2).
  - all nonlinearities are Sigmoid: tanh(g) = 2*sig(2g)-1 via weight
    pre-scaling, tanh(syn) = 2*sig(syn2)-1 with syn2 == 2*syn as the state.
  - per group-step: ONE sigmoid over the [128,128] gate bank, 3 DVE ops to
    syn2, ONE sigmoid for tanh(syn), 2 DVE ops to mem.
  - spikes for both groups in one scalar_tensor_tensor with accum_out
    giving the BN spike count for free (no big reduction).
  - BN batch stats: count AllReduce, folded into layer-2 weights on device.
  - mean-over-steps + fc as an accumulating K=128->M=8 matmul.
"""
import os
import numpy as np
import ml_dtypes

import concourse.bass as bass
import concourse.mybir as mybir
import concourse.tile as tile
from concourse.tile import add_dep_helper
from concourse.bass_utils import run_bass_kernel_spmd

BF = mybir.dt.bfloat16
F32 = mybir.dt.float32
AF = mybir.ActivationFunctionType
OP = mybir.AluOpType

NCORES = 8
B, T, C = 256, 512, 14
H = 128
CH = 32           # conv output channels
TC = T // NCORES  # 64 t-columns per core
TCH = TC // 2     # 32 per pipeline group
STEPS = int(os.environ.get("SLSTM_STEPS", B))
EPS = 1e-5


def _bf16(x):
    return np.asarray(x, np.float32).astype(ml_dtypes.bfloat16)


def _reorder_gates_cols(wt):
    # [*, 512] gate-major cols in torch order i,f,g,o -> (g,i,f,o), scale g by 2
    i, f, g, o = (wt[..., k * H:(k + 1) * H] for k in range(4))
    return np.concatenate([2.0 * g, i, f, o], axis=-1)


def build_kernel(thr1: float, thr2: float):
    # The spike compare (q - thr > spk01_prev) and the {0,1} record encoding
    # assume unit thresholds (true for this problem instance).
    assert thr1 == 1.0 and thr2 == 1.0, "kernel specialized for thr == 1.0"
    nc = bass.Bass()

    # ---- external I/O ----
    xt3_d = nc.dram_tensor("xt3", [85, B * TC], BF, kind="ExternalInput")
    wconv_d = nc.dram_tensor("wconv", [85, CH], BF, kind="ExternalInput")
    w1h_d = nc.dram_tensor("w1h", [CH, 4 * H], BF, kind="ExternalInput")
    whh1m2_d = nc.dram_tensor("whh1m2", [H, 4 * H], BF, kind="ExternalInput")
    whh1ng_d = nc.dram_tensor("whh1ng", [H, 4 * H], BF, kind="ExternalInput")
    w2t32_d = nc.dram_tensor("w2t32", [H, 4 * H], F32, kind="ExternalInput")
    w2tbf_d = nc.dram_tensor("w2tbf", [H, 4 * H], BF, kind="ExternalInput")
    whh2m2_d = nc.dram_tensor("whh2m2", [H, 4 * H], BF, kind="ExternalInput")
    whh2ng_d = nc.dram_tensor("whh2ng", [H, 4 * H], BF, kind="ExternalInput")
    fcwtn_d = nc.dram_tensor("fcwtn", [H, 8], BF, kind="ExternalInput")
    b2sum_d = nc.dram_tensor("b2sum", [1, 4 * H], F32, kind="ExternalInput")
    b1p_d = nc.dram_tensor("b1p", [4, H], BF, kind="ExternalInput")
    sel4_d = nc.dram_tensor("sel4", [4, 4 * TCH], BF, kind="ExternalInput")
    fcwt_d = nc.dram_tensor("fcwt", [H, 8], BF, kind="ExternalInput")
    fcb_d = nc.dram_tensor("fcb", [8, 1], F32, kind="ExternalInput")
    gamma_d = nc.dram_tensor("gamma", [H, 1], F32, kind="ExternalInput")
    beta_d = nc.dram_tensor("beta", [H, 1], F32, kind="ExternalInput")
    out_d = nc.dram_tensor("out", [8, TC], F32, kind="ExternalOutput")
    DBG = bool(int(os.environ.get("SLSTM_DEBUG", "0")))
    if DBG:
        s0s_dd = nc.dram_tensor("s0s_d", [CH, 4096], BF, kind="ExternalOutput")
        sb1_dd = nc.dram_tensor("sb1_d", [H, B * TC], BF, kind="ExternalOutput")
        cnt_dd = nc.dram_tensor("cnt_d", [H, 1], F32, kind="ExternalOutput")
        b2e_dd = nc.dram_tensor("b2e_d", [1, 4 * H], BF, kind="ExternalOutput")
        w2e_dd = nc.dram_tensor("w2e_d", [H, 4 * H], BF, kind="ExternalOutput")

    NQ = (B * TC) // 2048           # 8 quads, each [128, 512] (4 chunks stacked)

    with tile.TileContext(nc) as tc:
        import contextlib
        ctx = contextlib.ExitStack()
        with ctx:
            const = ctx.enter_context(tc.tile_pool(name="const", bufs=1))
            big = ctx.enter_context(tc.tile_pool(name="big", bufs=1))
            spool = ctx.enter_context(tc.tile_pool(name="spool", bufs=4))
            vpool = ctx.enter_context(tc.tile_pool(name="vpool", bufs=4))
            sypool = ctx.enter_context(tc.tile_pool(name="sypool", bufs=4))
            tpool = ctx.enter_context(tc.tile_pool(name="tpool", bufs=4))
            mpool = ctx.enter_context(tc.tile_pool(name="mpool", bufs=3))
            qpool = ctx.enter_context(tc.tile_pool(name="qpool", bufs=3))
            s2pool = ctx.enter_context(tc.tile_pool(name="s2pool", bufs=3))
            cpool = ctx.enter_context(
                tc.tile_pool(name="cpool", bufs=2, space="PSUM"))
            gapool = ctx.enter_context(
                tc.tile_pool(name="gapool", bufs=2, space="PSUM"))
            gbpool = ctx.enter_context(
                tc.tile_pool(name="gbpool", bufs=2, space="PSUM"))
            fpool = ctx.enter_context(
                tc.tile_pool(name="fpool", bufs=1, space="PSUM"))
            dram = ctx.enter_context(
                tc.tile_pool(name="dram", bufs=1, space="DRAM"))

            def load(pool, dt_, dram_t, shape):
                t_ = pool.tile(shape, dt_, name=dram_t.name + "_sb")
                nc.sync.dma_start(t_[:], dram_t[:])
                return t_

            wconv_sb = load(const, BF, wconv_d, [85, CH])
            xt3_sb = big.tile([85, B * TC], BF, name="xt3_sb")
            # stream input in 4 chunks so conv can start early
            for cch in range(4):
                sl = slice(cch * 4096, (cch + 1) * 4096)
                nc.sync.dma_start(xt3_sb[:, sl], xt3_d[:, sl])
            w1h_sb = load(const, BF, w1h_d, [CH, 4 * H])
            whh1m2_sb = load(const, BF, whh1m2_d, [H, 4 * H])
            whh1ng_sb = load(const, BF, whh1ng_d, [H, 4 * H])
            w2t32_sb = load(const, F32, w2t32_d, [H, 4 * H])
            w2tbf_sb = load(const, BF, w2tbf_d, [H, 4 * H])
            whh2m2_sb = load(const, BF, whh2m2_d, [H, 4 * H])
            whh2ng_sb = load(const, BF, whh2ng_d, [H, 4 * H])
            fcwtn_sb = load(const, BF, fcwtn_d, [H, 8])
            b2sum_sb = load(const, F32, b2sum_d, [1, 4 * H])
            b1p_sb = load(const, BF, b1p_d, [4, H])
            sel4_sb = load(const, BF, sel4_d, [4, 4 * TCH])
            fcwt_sb = load(const, BF, fcwt_d, [H, 8])
            fcb_sb = load(const, F32, fcb_d, [8, 1])
            gamma_sb = load(const, F32, gamma_d, [H, 1])
            beta_sb = load(const, F32, beta_d, [H, 1])

            sb1 = big.tile([H, B * TC], BF, name="sb1")      # layer-1 spikes
            nc.vector.memset(sb1[:], 0.0)
            zeros64 = const.tile([H, TC], BF, name="zeros64")
            nc.vector.memset(zeros64[:], 0.0)
            thrb1 = const.tile([H, TC], BF, name="thrb1")
            nc.vector.memset(thrb1[:], thr1)
            thrb2 = const.tile([H, TC], BF, name="thrb2")
            nc.vector.memset(thrb2[:], thr2)

            # ---- conv + sign spikes, fully upfront ----
            negone = const.tile([H, 1], F32, name="negone")
            nc.vector.memset(negone[:], -1.0)
            s0u = big.tile([CH, B * TC], BF, name="s0u")
            for c in range(4 * NQ):
                cp = cpool.tile([CH, 512], F32, name="convp", tag="convp")
                sl = slice(c * 512, (c + 1) * 512)
                nc.tensor.matmul(cp[:, :], wconv_sb[:, :], xt3_sb[:, sl],
                                 start=True, stop=True)
                nc.scalar.activation(s0u[:, sl], cp[:], AF.Sign,
                                     bias=negone[0:CH, :])

            # ---------------- the two-layer pipelined scan ----------------
            # State: sy' = syn/2 per group; spikes recorded as
            # sbar' = thr*(spike+1) in {thr, 2thr} (sbar'(-1) := thr).
            # mem(b) = q(b) - sbar'(b-1) + thr with q = 2*mp2 - sigma_o,
            # mp2 = sigma_o * sigma(4*sy'); mem is never materialized: the
            # recurrent matmul uses 2whh^T mp2 - whh^T sigma_o - whh^T sbar'
            # with +thr*whh^T*1 folded into the (host) bias.
            def _gsl(grp):
                return slice(0, TCH) if grp == "A" else slice(TCH, TC)

            def prefill(gp, layer, grp, b, close=False):
                """bias + input matmuls for step b of group grp -> new bank."""
                gb = gp.tile([H, 4 * TCH], F32, name=f"g{grp}", tag=f"g{grp}")
                bp = b1p_sb if layer == 1 else b2p_sb
                nc.tensor.matmul(gb[:, :], bp[:, :], sel4_sb[:, :],
                                 start=True, stop=False)
                if layer == 1:
                    off = b * TC + (0 if grp == "A" else TCH)
                    for g in range(4):
                        nc.tensor.matmul(
                            gb[:, g * TCH:(g + 1) * TCH],
                            w1h_sb[:, g * H:(g + 1) * H],
                            s0u[:, off:off + TCH],
                            start=False, stop=(close and g == 3))
                else:
                    off = b * TC + (0 if grp == "A" else TCH)
                    for g in range(4):
                        nc.tensor.matmul(
                            gb[:, g * TCH:(g + 1) * TCH],
                            w2eff_sb[:, g * H:(g + 1) * H],
                            sb1[:, off:off + TCH],
                            start=False, stop=(close and g == 3))
                return gb

            def sbar_mms(layer, gb, sbar2, grp, stop):
                """gates -= whh^T sbar'(b-2)  (sbar2: [H,TC] tile/slice)."""
                ng = whh1ng_sb if layer == 1 else whh2ng_sb
                sl = _gsl(grp)
                for g in range(4):
                    nc.tensor.matmul(gb[:, g * TCH:(g + 1) * TCH],
                                     ng[:, g * H:(g + 1) * H], sbar2[:, sl],
                                     start=False, stop=(stop and g == 3))

            def so_mms(layer, gb, S):
                """gates -= whh^T sigma_o(prev step of this group)."""
                ng = whh1ng_sb if layer == 1 else whh2ng_sb
                for g in range(4):
                    nc.tensor.matmul(gb[:, g * TCH:(g + 1) * TCH],
                                     ng[:, g * H:(g + 1) * H],
                                     S[:, 3 * TCH:4 * TCH],
                                     start=False, stop=False)

            def whh_mms(layer, gb, mp64p, grp):
                """gates += 2*whh^T mp2(b-1) — chain-critical, closes bank."""
                m2 = whh1m2_sb if layer == 1 else whh2m2_sb
                sl = _gsl(grp)
                last = None
                for g in range(4):
                    last = nc.tensor.matmul(gb[:, g * TCH:(g + 1) * TCH],
                                            m2[:, g * H:(g + 1) * H],
                                            mp64p[:, sl],
                                            start=False, stop=(g == 3))
                return last

            def emit_sig(gb, grp):
                S = spool.tile([H, 4 * TCH], BF, name=f"S{grp}", tag=f"S{grp}")
                nc.scalar.activation(S[:], gb[:], AF.Sigmoid)
                return S

            def emit_ts(sy, grp):
                # ts' = sigma(syn2)   (sy holds syn2 = 2*syn directly)
                ts = tpool.tile([H, TCH], BF, name=f"ts{grp}", tag=f"ts{grp}")
                nc.scalar.activation(ts[:], sy[:], AF.Sigmoid)
                return ts

            def dve_head(S, syp, grp):
                # fs = S_f*sy'(b-1) ; u2 = (S_g-0.5)*S_i ; sy' = u2+fs
                Sg, Si = S[:, 0:TCH], S[:, TCH:2 * TCH]
                Sf = S[:, 2 * TCH:3 * TCH]
                fs = vpool.tile([H, TCH], BF, name=f"fs{grp}", tag=f"fs{grp}")
                nc.vector.tensor_tensor(fs[:], Sf, syp[:], op=OP.mult)
                u2 = vpool.tile([H, TCH], BF, name=f"u2{grp}", tag=f"u2{grp}")
                nc.vector.scalar_tensor_tensor(u2[:], Sg, -0.5, Si,
                                               op0=OP.add, op1=OP.mult)
                sy = sypool.tile([H, TCH], BF, name=f"sy{grp}", tag=f"sy{grp}")
                syi = nc.vector.scalar_tensor_tensor(sy[:], u2[:], 4.0, fs[:],
                                                     op0=OP.mult, op1=OP.add)
                return sy, syi

            def emit_mp2(S, ts, mp64, grp):
                # mp2 = sigma_o * ts'  (chain tail — feeds whh of next step)
                nc.vector.tensor_tensor(mp64[:, _gsl(grp)], ts[:],
                                        S[:, 3 * TCH:4 * TCH], op=OP.mult)

            def run_layer(layer, thr, thrb):
                gp = {"A": gapool, "B": gbpool}

                def emit_q(mp64b, SAp, SBp):
                    """q = 2*mp2 - sigma_o (DVE, lands in the whh/sigma idle
                    window right after mp2B)."""
                    q64 = qpool.tile([H, TC], BF, name="q64", tag="q64")
                    nc.vector.scalar_tensor_tensor(
                        q64[:, 0:TCH], mp64b[:, 0:TCH], 2.0,
                        SAp[:, 3 * TCH:4 * TCH], op0=OP.mult, op1=OP.subtract)
                    nc.vector.scalar_tensor_tensor(
                        q64[:, TCH:TC], mp64b[:, TCH:TC], 2.0,
                        SBp[:, 3 * TCH:4 * TCH], op0=OP.mult, op1=OP.subtract)
                    return q64

                def emit_spk01(b, q64, recp):
                    """spike(b) = (q - spk01(b-1)) > thr, written straight to
                    the record (thr == 1; records are plain {0,1} spikes).
                    Two ops: STT-with-is_gt is silently broken on real HW."""
                    d = vpool.tile([H, TC], BF, name="dq", tag="dq")
                    nc.vector.tensor_tensor(d[:], q64[:], recp[:, :],
                                            op=OP.subtract)
                    if layer == 1:
                        dst = sb1[:, b * TC:(b + 1) * TC]
                        nc.vector.tensor_scalar(dst, d[:], thr, None, OP.is_gt)
                        return dst
                    dt_ = s2pool.tile([H, TC], BF, name="sb2", tag="sb2")
                    nc.vector.tensor_scalar(dt_[:], d[:], thr, None, OP.is_gt)
                    return dt_[:, :]


                def fc_mms(s, q64b, sbrec):
                    """fc accumulation for step s of layer 2."""
                    nc.tensor.matmul(fcp[:, :], fcwt_sb[:, :], q64b[:, :],
                                     start=(s == 0), stop=False)
                    if s > 0:
                        nc.tensor.matmul(fcp[:, :], fcwtn_sb[:, :],
                                         sbrec[:, :],
                                         start=False, stop=(s == STEPS - 1))
                    # stop flag handled by caller for the very last q-mm pair

                # step-0 banks: bias+input only (no recurrent terms)
                gbk = {}
                for g in ("A", "B"):
                    gbk[g] = prefill(gp[g], layer, g, 0, close=True)
                syp = {"A": zeros64[:, 0:TCH], "B": zeros64[:, 0:TCH]}
                mp64p = None        # mp2 tile of step b-1
                Sp = {}             # S tiles of step b (then b-1)
                tsBp = None
                sbp = zeros64[:, :]  # spk01(b-1) view ({0,1} records)
                q64p = None
                for b in range(STEPS):
                    nxt = {}
                    # --- finish step b-1: mp2B (DVE chain), q ---
                    if b > 0:
                        emit_mp2(Sp["B"], tsBp, mp64p, "B")
                        q64p = emit_q(mp64p, Sp["A"], Sp["B"])
                        if layer == 2:
                            fc_mms(b - 1, q64p, sbp)   # sbp = sbar'(b-2)
                    # --- PE: whh(b), bank(b+1) prep, fc(b-1) ---
                    if b > 0:
                        whh_mms(layer, gbk["A"], mp64p, "A")
                    if b + 1 < STEPS:
                        nxt["A"] = prefill(gp["A"], layer, "A", b + 1)
                    if b > 0:
                        whh_mms(layer, gbk["B"], mp64p, "B")
                    if b + 1 < STEPS:
                        nxt["B"] = prefill(gp["B"], layer, "B", b + 1)
                    # --- Act + DVE chain for step b; spike ops of step b-1
                    # gated on syA/syB so they fill the ts-roundtrip windows
                    SA = emit_sig(gbk["A"], "A")
                    if b + 1 < STEPS:
                        so_mms(layer, nxt["A"], SA)
                    mp64 = mpool.tile([H, TC], BF, name="mp64", tag="mp64")
                    syA, syAi = dve_head(SA, syp["A"], "A")
                    tsA = emit_ts(syA, "A")
                    emit_mp2(SA, tsA, mp64, "A")
                    SB = emit_sig(gbk["B"], "B")
                    if b + 1 < STEPS:
                        so_mms(layer, nxt["B"], SB)
                    syB, syBi = dve_head(SB, syp["B"], "B")
                    tsB = emit_ts(syB, "B")
                    # --- off-chain (late priority): spk01/record (b-1) ---
                    if b > 0:
                        sbp = emit_spk01(b - 1, q64p, sbp)
                    if 1 < b + 1 < STEPS:
                        sbar_mms(layer, nxt["A"], sbp, "A", stop=False)
                        sbar_mms(layer, nxt["B"], sbp, "B", stop=False)
                    syp = {"A": syA, "B": syB}
                    Sp, tsBp = {"A": SA, "B": SB}, tsB
                    mp64p = mp64
                    gbk = nxt
                # trailing: finish last step
                emit_mp2(Sp["B"], tsBp, mp64p, "B")
                q64p = emit_q(mp64p, Sp["A"], Sp["B"])
                if layer == 2:
                    nc.tensor.matmul(fcp[:, :], fcwt_sb[:, :], q64p[:, :],
                                     start=(STEPS == 1), stop=False)
                    nc.tensor.matmul(fcp[:, :], fcwtn_sb[:, :], sbp[:, :],
                                     start=False, stop=True)
                if layer == 1:
                    emit_spk01(STEPS - 1, q64p, sbp)

            # ---- phase A ----
            run_layer(1, thr1, thrb1)

            # ---- BN stats -> AllReduce -> fold into layer-2 weights ----
            cnt = const.tile([H, 1], F32, name="cnt")
            nc.vector.tensor_reduce(cnt[:], sb1[:, 0:STEPS * TC],
                                    axis=mybir.AxisListType.X, op=OP.add)
            cc_in = dram.tile([H, 1], F32, name="cc_in")
            cc_out = dram.tile([H, 1], F32, name="cc_out", addr_space="Shared")
            nc.sync.dma_start(cc_in[:], cnt[:])
            nc.gpsimd.collective_compute(
                "AllReduce", OP.add,
                replica_groups=[list(range(NCORES))],
                ins=[cc_in[:]], outs=[cc_out[:]])
            cntg = const.tile([H, 1], F32, name="cntg")
            nc.sync.dma_start(cntg[:], cc_out[:])

            p_t = const.tile([H, 1], F32, name="p_t")
            nc.vector.tensor_scalar(p_t[:], cntg[:], 1.0 / (B * T), None,
                                    OP.mult)
            q_t = const.tile([H, 1], F32, name="q_t")
            nc.vector.tensor_scalar(q_t[:], p_t[:], -1.0, 1.0, OP.mult, OP.add)
            var_t = const.tile([H, 1], F32, name="var_t")
            nc.vector.tensor_tensor(var_t[:], p_t[:], q_t[:], op=OP.mult)
            nc.vector.tensor_scalar(var_t[:], var_t[:], EPS, None, OP.add)
            sq_t = const.tile([H, 1], F32, name="sq_t")
            nc.scalar.activation(sq_t[:], var_t[:], AF.Sqrt, bias=0.0)
            rs_t = const.tile([H, 1], F32, name="rs_t")
            nc.vector.reciprocal(rs_t[:], sq_t[:])
            a_t = const.tile([H, 1], F32, name="a_t")
            nc.vector.tensor_tensor(a_t[:], gamma_sb[:], rs_t[:], op=OP.mult)
            pa_t = const.tile([H, 1], F32, name="pa_t")
            nc.vector.tensor_tensor(pa_t[:], p_t[:], a_t[:], op=OP.mult)
            c_t = const.tile([H, 1], F32, name="c_t")
            nc.vector.scalar_tensor_tensor(c_t[:], pa_t[:], -1.0, beta_sb[:],
                                           op0=OP.mult, op1=OP.add)
            cbf_t = const.tile([H, 1], BF, name="cbf_t")
            nc.vector.tensor_copy(cbf_t[:], c_t[:])

            w2eff_sb = const.tile([H, 4 * H], BF, name="w2eff")
            nc.vector.tensor_scalar(w2eff_sb[:], w2t32_sb[:], a_t[:], None,
                                    OP.mult)

            bp = cpool.tile([1, 4 * H], F32, name="biasp", tag="convp")
            nc.tensor.matmul(bp[:, :], cbf_t[:, :], w2tbf_sb[:, :],
                             start=True, stop=True)
            b2eff_sb = const.tile([1, 4 * H], BF, name="b2eff")
            nc.vector.tensor_tensor(b2eff_sb[:], b2sum_sb[:], bp[:, :],
                                    op=OP.add)
            # reshape [1,512] -> [4,128] across partitions via linear DRAM
            b2lin = dram.tile([4, H], BF, name="b2lin")
            nc.sync.dma_start(b2lin[:].rearrange("a b -> () (a b)"), b2eff_sb[:])
            b2p_sb = const.tile([4, H], BF, name="b2p")
            nc.sync.dma_start(b2p_sb[:], b2lin[:])

            # ---- phase B ----
            fcp = fpool.tile([8, TC], F32, name="fcp", tag="fcp")
            run_layer(2, thr2, thrb2)

            out_sb = const.tile([8, TC], F32, name="out_sb")
            nc.scalar.activation(out_sb[:], fcp[:, :], AF.Identity,
                                 bias=fcb_sb[:])
            nc.sync.dma_start(out_d[:], out_sb[:])

            if DBG:
                nc.sync.dma_start(s0s_dd[:], s0u[:, 0:4096])
                nc.sync.dma_start(sb1_dd[:], sb1[:])
                nc.sync.dma_start(cnt_dd[:], cnt[:])
                nc.sync.dma_start(b2e_dd[:], b2eff_sb[:])
                nc.sync.dma_start(w2e_dd[:], w2eff_sb[:])

    # _strip_self_waits is simulator-safe but races on real silicon
    # (same-engine RAW needs the Tile-emitted sems); keep it off.
    if int(os.environ.get("SLSTM_STRIP", "0")):
        _strip_self_waits(nc)
    _split_mm_waits(nc)
    return nc


def _strip_self_waits(nc):
    """Remove semaphore waits where an instruction waits on its OWN engine's
    tick semaphore for a tick it is guaranteed to be behind (in-order engine
    pipes make same-engine RAW/WAR safe without sems on HW; TimelineSim
    charges ~95ns per such wait)."""
    CORE = {mybir.EngineType.DVE, mybir.EngineType.Activation,
            mybir.EngineType.PE, mybir.EngineType.Pool}

    def _is(v, name):
        return str(v).replace("-", "_").endswith(name)

    # map sem id -> set of engines that increment it
    updaters = {}
    for fn in nc.m.functions:
        for blk in fn.blocks:
            for inst in blk.instructions:
                si = getattr(inst, "sync_info", None)
                if si is None:
                    continue
                for u in si.on_update:
                    if _is(u.sync_type, "semaphore") and _is(u.update_mode,
                                                            "sem_inc"):
                        updaters.setdefault(u.id, set()).add(inst.engine)
    tick_sems = {}   # engine -> sem id exclusively updated by that engine
    for sid, engs in updaters.items():
        if len(engs) == 1:
            e = next(iter(engs))
            if e in CORE:
                tick_sems.setdefault(e, set()).add(sid)
    # walk in order, tracking per-sem counts of already-issued increments
    count = {}
    nstrip = 0
    for fn in nc.m.functions:
        for blk in fn.blocks:
            for inst in blk.instructions:
                si = getattr(inst, "sync_info", None)
                if si is None:
                    continue
                if si.on_wait and inst.engine in CORE:
                    own = tick_sems.get(inst.engine, set())
                    keep = []
                    for w in si.on_wait:
                        if (_is(w.sync_type, "semaphore") and w.id in own
                                and _is(w.wait_mode, "sem_ge_imm")
                                and w.wait_value is not None
                                and w.wait_value <= count.get(w.id, 0)):
                            nstrip += 1
                            continue
                        keep.append(w)
                    si.on_wait = keep
                for u in si.on_update:
                    if _is(u.sync_type, "semaphore"):
                        count[u.id] = count.get(u.id, 0) + 1


def _split_mm_waits(nc):
    """The S3D3 ISA structs carry only one sync-wait slot; move extra
    Tile-assigned waits onto preceding NoOps. Keep the wait that is
    predicted to fire LAST (largest deficit vs emitted tick counts) on the
    instruction itself, so the NoOps' waits are already satisfied and don't
    stall the sequencer on the critical chain."""
    count = {}
    for fn in nc.m.functions:
        for blk in fn.blocks:
            out = []
            for inst in blk.instructions:
                si = getattr(inst, "sync_info", None)
                if (not isinstance(inst, (mybir.InstEventSemaphore,
                                          mybir.InstAllEngineBarrier,
                                          mybir.InstNoOp))
                        and si is not None and si.on_wait
                        and len(si.on_wait) > 1):
                    def deficit(w):
                        if w.wait_value is None:
                            return 1 << 30
                        return w.wait_value - count.get(w.id, 0)
                    keep = max(si.on_wait, key=deficit)
                    rest = [w for w in si.on_wait if w is not keep]
                    for j, w in enumerate(rest):
                        nop = mybir.InstNoOp(name=f"{inst.name}-wsplit{j}",
                                             ins=[], outs=[])
                        nop.engine = inst.engine
                        nop.sync_info = mybir.SyncInfo(on_wait=[w],
                                                       on_update=[])
                        out.append(nop)
                    si.on_wait = [keep]
                out.append(inst)
                if si is not None:
                    for u in si.on_update:
                        count[u.id] = count.get(u.id, 0) + 1
            blk.instructions[:] = out


def _host_inputs(x, conv_w, conv_b, w_ih1, w_hh1, b_ih1, b_hh1, thr1, thr2,
                 w_ih2, w_hh2, b_ih2, b_hh2, bn_gamma, bn_beta, fc_w, fc_b):
    f32 = np.float32
    xp = np.pad(np.asarray(x, f32), ((0, 0), (1, 1), (0, 0)))  # [B, T+2, C]
    common = {}
    w3t = np.concatenate([conv_w[:, :, k].T for k in range(3)], axis=0)
    common["wconv"] = _bf16(np.concatenate(
        [w3t, w3t, np.asarray(conv_b, f32)[None, :]], axis=0))
    # layer-1 weights on {-1,+1} conv spikes: w/2, 4x row-stacked
    common["w1h"] = _bf16(
        _reorder_gates_cols(0.5 * np.asarray(w_ih1, f32).T))    # [32, 512]
    whh1r = _reorder_gates_cols(np.asarray(w_hh1, f32).T)       # [128, 512]
    common["whh1m2"] = _bf16(2.0 * whh1r)
    common["whh1ng"] = _bf16(-float(thr1) * whh1r)
    # bias: +w@1/2 (sign spikes), +thr1*whh^T 1 (sbar' record offset)
    bias1 = (np.asarray(b_ih1, f32) + np.asarray(b_hh1, f32)
             + 0.5 * np.asarray(w_ih1, f32).sum(axis=1))
    common["b1p"] = _bf16(_reorder_gates_cols(bias1[None, :]).reshape(4, H))
    sel = np.zeros((4, 4 * TCH), f32)
    for g in range(4):
        sel[g, g * TCH:(g + 1) * TCH] = 1.0
    common["sel4"] = _bf16(sel)
    w2t = _reorder_gates_cols(np.asarray(w_ih2, f32).T)         # [128, 512]
    common["w2t32"] = np.ascontiguousarray(float(thr1) * w2t, f32)
    common["w2tbf"] = _bf16(w2t)
    whh2r = _reorder_gates_cols(np.asarray(w_hh2, f32).T)
    common["whh2m2"] = _bf16(2.0 * whh2r)
    common["whh2ng"] = _bf16(-float(thr2) * whh2r)
    common["b2sum"] = np.ascontiguousarray(
        _reorder_gates_cols((np.asarray(b_ih2) + np.asarray(b_hh2))[None, :]),
        f32)
    common["fcwt"] = _bf16(np.asarray(fc_w, f32).T / STEPS)
    common["fcwtn"] = _bf16(-float(thr2) * np.asarray(fc_w, f32).T / STEPS)
    common["fcb"] = np.ascontiguousarray(np.asarray(fc_b, f32)[:, None], f32)
    common["gamma"] = np.ascontiguousarray(
        np.asarray(bn_gamma, f32)[:, None], f32)
    common["beta"] = np.ascontiguousarray(
        np.asarray(bn_beta, f32)[:, None], f32)

    in_maps = []
    for k in range(NCORES):
        xw = xp[:, TC * k: TC * k + TC + 2, :]                  # [B, 66, C]
        taps = [xw[:, kk:kk + TC, :].transpose(2, 0, 1).reshape(C, B * TC)
                for kk in range(3)]
        arr = np.concatenate(taps, axis=0)                      # [42, B*64]
        hi = arr.astype(ml_dtypes.bfloat16)
        lo = (arr - hi.astype(f32)).astype(ml_dtypes.bfloat16)
        ones = np.ones((1, B * TC), ml_dtypes.bfloat16)
        m = dict(common)
        m["xt3"] = np.ascontiguousarray(np.concatenate([hi, lo, ones], axis=0))
        in_maps.append(m)
    return in_maps


_CACHE = {}


def kernel(x, conv_w, conv_b, w_ih1, w_hh1, b_ih1, b_hh1, thr1,
           w_ih2, w_hh2, b_ih2, b_hh2, thr2, bn_gamma, bn_beta,
           fc_w, fc_b):
    thr1 = float(np.asarray(thr1)); thr2 = float(np.asarray(thr2))
    key = (thr1, thr2)
    if key not in _CACHE:
        _CACHE[key] = build_kernel(thr1, thr2)
    nc = _CACHE[key]
    in_maps = _host_inputs(x, conv_w, conv_b, w_ih1, w_hh1, b_ih1, b_hh1,
                           thr1, thr2, w_ih2, w_hh2, b_ih2, b_hh2,
                           bn_gamma, bn_beta, fc_w, fc_b)
    res = run_bass_kernel_spmd(nc, in_maps, core_ids=list(range(NCORES)),
                               trace=bool(int(os.environ.get("SLSTM_TRACE", "0"))))
    outT = np.concatenate([r["out"] for r in res.results], axis=1)  # [8, 512]
    if res.exec_time_ns is not None:
        kernel.last_exec_time_ns = res.exec_time_ns
    return np.ascontiguousarray(outT.T.astype(np.float32))
